# revision 36
# baseline (speedup 1.0000x reference)
"""Additive (Bahdanau) attention on Trainium2, 8-core SPMD, data-parallel over batch.

Raw-Bass implementation (explicit semaphores; the Tile scheduler attaches >1
sync-wait command per instruction, which this walrus build cannot codegen —
raw bass emits standalone wait_ge instructions instead).

Per core (4 local batches), one streaming pass over enc_seq:
  encT tile [128h, 4hc, 512t]  <- DMA xbar transpose of enc_bf16 (DRAM)
  enc  tile [128t, 4sc, 512h]  <- DMA xbar transpose of encTT_bf16 (DRAM)
  hT[k,t] = sum_h WhT[h,k] encT[h,t]    (bf16 MMs, f32 PSUM, 4 k-chunks)
  g = tanh(hT + s[k])                   (ACT, per-partition bias, bf16 out)
  e_col[t] = sum_k g[k,t] v[k]          (PE, g stationary, e in column form)
  p = exp(e) * mask                     (no max subtraction: |e| <= sum|v| ~ 23)
  ctx_u[b] += p_chunk.T @ enc_chunk     (PE chain per tile, DVE-accumulated)
Host: s = dec @ Ws.T pre-projection, bf16 layout prep, final softmax
normalization (a = p/sum(p), ctx = ctx_u/sum(p)), and the p de-transpose.
"""

import numpy as np
import ml_dtypes
from contextlib import ExitStack

HID = 512
T = 4096
B = 32
N_CORES = 8
B_LOC = B // N_CORES          # 4 batches per core
T_TILE = 512
N_J = T // T_TILE             # 8 t-tiles per batch
N_SC = T_TILE // 128          # 4 sub-chunks per tile
N_HC = HID // 128             # 4 h-chunks
N_KC = HID // 128             # 4 k-chunks
N_COLS = B_LOC * N_J * N_SC   # 128 p-columns per core
N_TILES = N_J * B_LOC         # 32 (b,j) tiles per core

_cache = {}


def _build_nc():
    import concourse.bass as bass
    from concourse import mybir

    f32 = mybir.dt.float32
    bf16 = mybir.dt.bfloat16
    AF = mybir.ActivationFunctionType
    ts = bass.ts

    nc = bass.Bass()
    enc = nc.dram_tensor("enc", [B_LOC, T, HID], bf16, kind="ExternalInput")
    encTT = nc.dram_tensor("encTT", [B_LOC, HID, T], bf16, kind="ExternalInput")
    wh = nc.dram_tensor("wh", [HID, HID], bf16, kind="ExternalInput")
    vp = nc.dram_tensor("vp", [16, 128], bf16, kind="ExternalInput")
    sp = nc.dram_tensor("sp", [16, HID], bf16, kind="ExternalInput")
    maskT = nc.dram_tensor("maskT", [N_COLS, 128], bf16, kind="ExternalInput")
    pcol = nc.dram_tensor("pcol", [128, N_COLS], f32, kind="ExternalOutput")
    ctxu = nc.dram_tensor("ctxu", [1, B_LOC * HID], f32, kind="ExternalOutput")

    tiles = [(j, b) for j in range(N_J) for b in range(B_LOC)]

    with ExitStack() as ex:
        ec = ex.enter_context
        whT = ec(nc.sbuf_tensor("whT", [128, N_HC, HID], bf16))
        vt = ec(nc.sbuf_tensor("vt", [128, 16], bf16))
        sT = ec(nc.sbuf_tensor("sT", [128, N_KC, 16], bf16))
        maskc = ec(nc.sbuf_tensor("maskc", [128, N_COLS], bf16))
        pcol_sb = ec(nc.sbuf_tensor("pcol_sb", [128, N_COLS], f32))
        ctxall = ec(nc.sbuf_tensor("ctxall", [1, B_LOC + 1, HID], f32))
        enc_sbs = [ec(nc.sbuf_tensor(f"encsb{k}", [128, N_SC, HID], bf16)) for k in range(3)]
        encTs = [ec(nc.sbuf_tensor(f"encT{k}", [128, N_HC, T_TILE], bf16)) for k in range(3)]
        gs = [ec(nc.sbuf_tensor(f"g{k}", [128, T_TILE], bf16)) for k in range(6)]
        ptmps = [ec(nc.sbuf_tensor(f"ptmp{k}", [128, N_SC], f32)) for k in range(2)]
        junk = ec(nc.sbuf_tensor("junk", [1, 4], f32))
        pbs = [ec(nc.sbuf_tensor(f"pb{k}", [128, N_SC], bf16)) for k in range(4)]

        hps = [ec(nc.psum_tensor(f"hps{k}", [128, T_TILE], f32)) for k in range(2)]
        # one PSUM bank per e-column: start=True clears accumulation state at
        # bank granularity, so per-column chains must not share a bank
        es = [ec(nc.psum_tensor(f"es{k}", [128, 1], f32)) for k in range(N_SC)]
        # ctx: single-chain group per tile (start..stop), DVE-accumulated into
        # SBUF. Two alternating banks: start=True clears state bank-wide, so
        # chain it+1 must not share a bank with chain it (whose row the DVE
        # still has to read), and DVE must never read a bank PE is writing.
        cpss = [ec(nc.psum_tensor(f"cps{k}", [128, HID], f32)) for k in range(2)]

        s_const = ec(nc.semaphore(name="s_const"))
        s_enc = ec(nc.semaphore(name="s_enc"))
        s_encT = ec(nc.semaphore(name="s_encT"))
        s_h = ec(nc.semaphore(name="s_h"))
        s_g = ec(nc.semaphore(name="s_g"))
        s_edone = ec(nc.semaphore(name="s_edone"))
        s_x = ec(nc.semaphore(name="s_x"))
        s_pm = ec(nc.semaphore(name="s_pm"))
        s_p = ec(nc.semaphore(name="s_p"))
        s_ctx = ec(nc.semaphore(name="s_ctx"))
        s_cadd = ec(nc.semaphore(name="s_cadd"))
        s_out = ec(nc.semaphore(name="s_out"))

        block = ec(nc.Block())

        @block.sync
        def _(sync):
            sync.dma_start_transpose(whT[:, :, :], wh[:, :]).then_inc(s_const, 16)
            sync.dma_start_transpose(vt[:, :], vp[:, :]).then_inc(s_const, 16)
            sync.dma_start_transpose(sT[:, :, :], sp[:, :]).then_inc(s_const, 16)
            sync.dma_start_transpose(maskc[:, :], maskT[:, :]).then_inc(s_const, 16)
            for i, (j, b) in enumerate(tiles):
                if i >= 3:
                    sync.wait_ge(s_ctx, i - 2)  # enc_sb slot: ctx-MMs of i-3 done
                sync.dma_start_transpose(
                    enc_sbs[i % 3][:, :, :], encTT[b, :, ts(j, T_TILE)]
                ).then_inc(s_enc, 16)
                if i >= 3:
                    sync.wait_ge(s_h, 4 * (i - 3) + 4)  # encT slot: h-MMs of i-3 done
                sync.dma_start_transpose(
                    encTs[i % 3][:, :, :], enc[b, ts(j, T_TILE), :]
                ).then_inc(s_encT, 16)
            # redo loads for tiles 0 and 1 (their first-pass ctx chains are
            # re-executed at the end; see ctx redo below)
            sync.wait_ge(s_ctx, N_TILES)
            for rd in range(2):
                jj, bb = tiles[rd]
                sync.dma_start_transpose(
                    enc_sbs[rd][:, :, :], encTT[bb, :, ts(jj, T_TILE)]
                ).then_inc(s_enc, 16)
            sync.wait_ge(s_enc, 16 * (N_TILES + 2))  # xposes done before regular DMAs
            sync.wait_ge(s_pm, N_TILES)
            sync.dma_start(pcol[:, :], pcol_sb[:, :]).then_inc(s_out, 16)
            sync.wait_ge(s_cadd, N_TILES + 2)
            sync.dma_start(ctxu[0:1, :], ctxall[0:1, :B_LOC, :]).then_inc(s_out, 16)
            sync.wait_ge(s_out, 32)

        @block.tensor
        def _(tensor):
            def hgroup(i, kc):
                idx = 4 * i + kc
                if idx >= 2:
                    tensor.wait_ge(s_g, idx - 1)  # hps slot consumed by tanh(idx-2)
                mm = None
                for hc in range(N_HC):
                    mm = tensor.matmul(
                        hps[idx % 2][:, :],
                        whT[:, hc, ts(kc, 128)],
                        encTs[i % 3][:, hc, :],
                        start=(hc == 0),
                        stop=(hc == N_HC - 1),
                        skip_group_check=True,
                    )
                mm.then_inc(s_h, 1)

            def egroup(i, kc):
                idx = 4 * i + kc
                if kc == 0 and i >= 1:
                    tensor.wait_ge(s_x, i)  # e banks: exp of i-1 done
                tensor.wait_ge(s_g, idx + 1)  # g(idx) ready
                mm = None
                for sc in range(N_SC):
                    mm = tensor.matmul(
                        es[sc][:, :],
                        gs[idx % 6][:, ts(sc, 128)],
                        vt[:, kc : kc + 1],
                        start=(kc == 0),
                        stop=(kc == N_KC - 1),
                        skip_group_check=True,
                    )
                mm.then_inc(s_edone, 1)

            def ctxgroup(it):
                jj, bb = tiles[it]
                tensor.wait_ge(s_p, it + 1)       # pb(it) ready
                tensor.wait_ge(s_enc, 16 * (it + 1))  # enc_sb(it) loaded
                if it >= 1:
                    # strict alternation with the DVE adds: chains launched
                    # with less delay than this produce corrupt PSUM output
                    tensor.wait_ge(s_cadd, it)
                mm = None
                for sc in range(N_SC):
                    mm = tensor.matmul(
                        cpss[it % 2][0:1, :],
                        pbs[it % 4][:, sc : sc + 1],
                        enc_sbs[it % 3][:, sc, :],
                        start=(sc == 0),
                        stop=(sc == N_SC - 1),
                        skip_group_check=True,
                    )
                mm.then_inc(s_ctx, 1)

            tensor.wait_ge(s_const, 64)
            # warm up the ctx PSUM banks: the first M=1 chain written to a
            # cold bank reads back empty on HW; make the real chains non-first
            for k in range(2):
                tensor.matmul(
                    cpss[k][0:1, :], vt[:, 0:1], whT[:, 0, :],
                    start=True, stop=True, skip_group_check=True,
                )
            for i, (j, b) in enumerate(tiles):
                tensor.wait_ge(s_encT, 16 * (i + 1))
                hgroup(i, 0)
                hgroup(i, 1)
                if i >= 1:
                    ctxgroup(i - 1)
                egroup(i, 0)
                hgroup(i, 2)
                egroup(i, 1)
                hgroup(i, 3)
                egroup(i, 2)
                egroup(i, 3)
            ctxgroup(N_TILES - 1)
            # dummy MM: extra s_ctx tick for the final add's drain margin
            tensor.matmul(
                hps[0][:, 0:1], whT[:, 0, 0:128], vt[:, 0:1],
                start=True, stop=True, skip_group_check=True,
            ).then_inc(s_ctx, 1)
            # redo chains for tiles 0/1: their first-pass results are
            # corrupted (earliest chains race something unidentified); chains
            # executed late are reliably correct
            for rd in range(2):
                tensor.wait_ge(s_p, N_TILES + 1 + rd)
                tensor.wait_ge(s_enc, 16 * (N_TILES + 1 + rd))
                tensor.wait_ge(s_cadd, N_TILES)  # all loop adds done
                mm = None
                for sc in range(N_SC):
                    mm = tensor.matmul(
                        cpss[rd][0:1, :],
                        pbs[rd][:, sc : sc + 1],
                        enc_sbs[rd][:, sc, :],
                        start=(sc == 0),
                        stop=(sc == N_SC - 1),
                        skip_group_check=True,
                    )
                mm.then_inc(s_ctx, 1)
            # trailing dummy for the last redo add's margin
            tensor.matmul(
                hps[0][:, 0:1], whT[:, 0, 0:128], vt[:, 0:1],
                start=True, stop=True, skip_group_check=True,
            ).then_inc(s_ctx, 1)

        @block.scalar
        def _(scalar):
            scalar.wait_ge(s_const, 64)
            for i, (j, b) in enumerate(tiles):
                for kc in range(N_KC):
                    idx = 4 * i + kc
                    if idx >= 6:
                        scalar.wait_ge(s_edone, idx - 5)  # g slot: e-MMs of idx-6 done
                    scalar.wait_ge(s_h, idx + 1)
                    scalar.activation(
                        gs[idx % 6][:, :],
                        hps[idx % 2][:, :],
                        AF.Tanh,
                        bias=sT[:, kc, b : b + 1],
                        scale=1.0,
                    ).then_inc(s_g, 1)
                if i >= 2:
                    scalar.wait_ge(s_pm, i - 1)  # ptmp slot: mul of i-2 done
                scalar.wait_ge(s_edone, 4 * i + 4)
                for sc in range(N_SC):
                    inst = scalar.activation(
                        ptmps[i % 2][:, sc : sc + 1], es[sc][:, :], AF.Exp
                    )
                inst.then_inc(s_x, 1)

        @block.vector
        def _(vector):
            vector.memset(ctxall[:, :, :], 0.0)
            vector.wait_ge(s_const, 64)

            def ctxadd(it):
                jj, bb = tiles[it]
                # tiles 0/1 race something at kernel start and are redone at
                # the end; park their corrupt first-pass adds in a trash row
                seg = bb if it >= 2 else B_LOC
                vector.wait_ge(s_ctx, it + 1)
                # s_ctx can fire before the chain's PSUM drain lands; also wait
                # for a later PE event (h2 of the next tile) as drain margin
                vector.wait_ge(s_h, min(4 * (it + 1) + 3, 4 * N_TILES))
                vector.tensor_add(
                    ctxall[0:1, seg, :],
                    ctxall[0:1, seg, :],
                    cpss[it % 2][0:1, :],
                ).then_inc(s_cadd, 1)

            for i, (j, b) in enumerate(tiles):
                cb = b * (N_J * N_SC) + j * N_SC
                vector.wait_ge(s_x, i + 1)
                vector.tensor_mul(
                    pcol_sb[:, cb : cb + N_SC],
                    ptmps[i % 2][:, :],
                    maskc[:, cb : cb + N_SC],
                ).then_inc(s_pm, 1)
                if i >= 2:
                    vector.wait_ge(s_ctx, i - 1)  # pb slot: ctx-MMs of i-2 done
                vector.tensor_copy(
                    out=pbs[i % 4][:, :], in_=pcol_sb[:, cb : cb + N_SC]
                )
                # s_p signalled by a trailing op: a DVE op issues only after
                # the previous op's pipe drains, so pb's writes are visible
                vector.memset(junk[0:1, :], 0.0).then_inc(s_p, 1)
                if i >= 1:
                    ctxadd(i - 1)
            ctxadd(N_TILES - 1)
            # redo: recopy p for tiles 0/1 from pcol_sb, then add the redone
            # chains (s_ctx ticks: 33 = dummy, 34/35 = redo chains, 36 = dummy)
            for rd in range(2):
                jr, br = tiles[rd]
                cbr = br * (N_J * N_SC) + jr * N_SC
                vector.tensor_copy(
                    out=pbs[rd][:, :], in_=pcol_sb[:, cbr : cbr + N_SC]
                )
                vector.memset(junk[0:1, :], 0.0).then_inc(s_p, 1)
            for rd in range(2):
                jr, br = tiles[rd]
                vector.wait_ge(s_ctx, N_TILES + 3 + rd)
                vector.tensor_add(
                    ctxall[0:1, br, :],
                    ctxall[0:1, br, :],
                    cpss[rd][0:1, :],
                ).then_inc(s_cadd, 1)

    return nc


def _prep_shared(W_h, W_s, v, dec_state):
    bf16 = ml_dtypes.bfloat16
    wh_in = np.ascontiguousarray(W_h).astype(bf16)
    vp_in = np.zeros((16, 128), dtype=bf16)
    vp_in[:N_KC] = v.reshape(N_KC, 128).astype(bf16)
    s = (dec_state.astype(np.float64) @ W_s.T.astype(np.float64)).astype(np.float32)
    return wh_in, vp_in, s


def kernel(enc_seq, enc_mask, dec_state, W_h, W_s, v):
    from concourse.bass_utils import run_bass_kernel_spmd

    enc_seq = np.asarray(enc_seq)
    enc_mask = np.asarray(enc_mask)
    dec_state = np.asarray(dec_state)
    W_h = np.asarray(W_h)
    W_s = np.asarray(W_s)
    v = np.asarray(v)
    bf16 = ml_dtypes.bfloat16

    if "nc" not in _cache:
        _cache["nc"] = _build_nc()
    nc = _cache["nc"]

    wh_in, vp_in, s = _prep_shared(W_h, W_s, v, dec_state)
    enc_bf = enc_seq.astype(bf16)                               # [B, T, H]
    encTT_bf = np.ascontiguousarray(enc_bf.transpose(0, 2, 1))  # [B, H, T]

    in_maps = []
    for core in range(N_CORES):
        bs = slice(core * B_LOC, (core + 1) * B_LOC)
        sp_in = np.zeros((16, HID), dtype=bf16)
        sp_in[:B_LOC] = s[bs].astype(bf16)
        # maskT[c, p] = mask[b, j*512 + sc*128 + p],  c = b*32 + j*4 + sc
        mk = (
            enc_mask[bs]
            .reshape(B_LOC, N_J, N_SC, 128)
            .reshape(N_COLS, 128)
            .astype(bf16)
        )
        in_maps.append(
            {
                "enc": np.ascontiguousarray(enc_bf[bs]),
                "encTT": np.ascontiguousarray(encTT_bf[bs]),
                "wh": wh_in,
                "vp": vp_in,
                "sp": sp_in,
                "maskT": np.ascontiguousarray(mk),
            }
        )

    res = run_bass_kernel_spmd(nc, in_maps, core_ids=list(range(N_CORES)))

    ctx = np.empty((B, HID), np.float32)
    a = np.empty((B, T), np.float32)
    for core in range(N_CORES):
        r = res.results[core]
        p_un = (
            r["pcol"]
            .reshape(128, B_LOC, N_J, N_SC)
            .transpose(1, 2, 3, 0)
            .reshape(B_LOC, T)
            .astype(np.float64)
        )
        denom = p_un.sum(axis=1, keepdims=True)
        bs = slice(core * B_LOC, (core + 1) * B_LOC)
        a[bs] = (p_un / denom).astype(np.float32)
        ctx[bs] = (r["ctxu"].reshape(B_LOC, HID).astype(np.float64) / denom).astype(
            np.float32
        )
    return ctx, a


# revision 38
# speedup vs baseline: 1.0572x; 1.0572x over previous
"""Additive (Bahdanau) attention on Trainium2, 8-core SPMD, data-parallel over batch.

Raw-Bass implementation (explicit semaphores; the Tile scheduler attaches >1
sync-wait command per instruction, which this walrus build cannot codegen —
raw bass emits standalone wait_ge instructions instead).

Per core (4 local batches), one streaming pass over enc_seq:
  encT tile [128h, 4hc, 512t]  <- DMA xbar transpose of enc_bf16 (DRAM)
  enc  tile [128t, 4sc, 512h]  <- DMA xbar transpose of encTT_bf16 (DRAM)
  hT[k,t] = sum_h WhT[h,k] encT[h,t]    (bf16 MMs, f32 PSUM, 4 k-chunks)
  g = tanh(hT + s[k])                   (ACT, per-partition bias, bf16 out)
  e_col[t] = sum_k g[k,t] v[k]          (PE, g stationary, e in column form)
  p = exp(e) * mask                     (no max subtraction: |e| <= sum|v| ~ 23)
  ctx_u[b] += p_chunk.T @ enc_chunk     (PE chain per tile, DVE-accumulated)
Host: s = dec @ Ws.T pre-projection, bf16 layout prep, final softmax
normalization (a = p/sum(p), ctx = ctx_u/sum(p)), and the p de-transpose.
"""

import numpy as np
import ml_dtypes
from contextlib import ExitStack

HID = 512
T = 4096
B = 32
N_CORES = 8
B_LOC = B // N_CORES          # 4 batches per core
T_TILE = 512
N_J = T // T_TILE             # 8 t-tiles per batch
N_SC = T_TILE // 128          # 4 sub-chunks per tile
N_HC = HID // 128             # 4 h-chunks
N_KC = HID // 128             # 4 k-chunks
N_COLS = B_LOC * N_J * N_SC   # 128 p-columns per core
N_TILES = N_J * B_LOC         # 32 (b,j) tiles per core

_cache = {}


def _build_nc():
    import concourse.bass as bass
    from concourse import mybir

    f32 = mybir.dt.float32
    bf16 = mybir.dt.bfloat16
    AF = mybir.ActivationFunctionType
    ts = bass.ts

    nc = bass.Bass()
    enc = nc.dram_tensor("enc", [B_LOC, T, HID], bf16, kind="ExternalInput")
    encTT = nc.dram_tensor("encTT", [B_LOC, HID, T], bf16, kind="ExternalInput")
    wh = nc.dram_tensor("wh", [HID, HID], bf16, kind="ExternalInput")
    vp = nc.dram_tensor("vp", [16, 128], bf16, kind="ExternalInput")
    sp = nc.dram_tensor("sp", [16, HID], bf16, kind="ExternalInput")
    maskT = nc.dram_tensor("maskT", [N_COLS, 128], bf16, kind="ExternalInput")
    pcol = nc.dram_tensor("pcol", [128, N_COLS], f32, kind="ExternalOutput")
    ctxu = nc.dram_tensor("ctxu", [1, B_LOC * HID], f32, kind="ExternalOutput")

    tiles = [(j, b) for j in range(N_J) for b in range(B_LOC)]

    with ExitStack() as ex:
        ec = ex.enter_context
        whT = ec(nc.sbuf_tensor("whT", [128, N_HC, HID], bf16))
        vt = ec(nc.sbuf_tensor("vt", [128, 16], bf16))
        sT = ec(nc.sbuf_tensor("sT", [128, N_KC, 16], bf16))
        maskc = ec(nc.sbuf_tensor("maskc", [128, N_COLS], bf16))
        pcol_sb = ec(nc.sbuf_tensor("pcol_sb", [128, N_COLS], f32))
        ctxall = ec(nc.sbuf_tensor("ctxall", [1, B_LOC + 1, HID], f32))
        enc_sbs = [ec(nc.sbuf_tensor(f"encsb{k}", [128, N_SC, HID], bf16)) for k in range(4)]
        encTs = [ec(nc.sbuf_tensor(f"encT{k}", [128, N_HC, T_TILE], bf16)) for k in range(3)]
        gs = [ec(nc.sbuf_tensor(f"g{k}", [128, T_TILE], bf16)) for k in range(8)]
        ptmps = [ec(nc.sbuf_tensor(f"ptmp{k}", [128, N_SC], f32)) for k in range(2)]
        junk = ec(nc.sbuf_tensor("junk", [1, 4], f32))
        pbs = [ec(nc.sbuf_tensor(f"pb{k}", [128, N_SC], bf16)) for k in range(4)]

        hps = [ec(nc.psum_tensor(f"hps{k}", [128, T_TILE], f32)) for k in range(2)]
        # one PSUM bank per e-column: start=True clears accumulation state at
        # bank granularity, so per-column chains must not share a bank
        es = [ec(nc.psum_tensor(f"es{k}", [128, 1], f32)) for k in range(N_SC)]
        # ctx: single-chain group per tile (start..stop), DVE-accumulated into
        # SBUF. Two alternating banks: start=True clears state bank-wide, so
        # chain it+1 must not share a bank with chain it (whose row the DVE
        # still has to read), and DVE must never read a bank PE is writing.
        cpss = [ec(nc.psum_tensor(f"cps{k}", [128, HID], f32)) for k in range(2)]

        s_const = ec(nc.semaphore(name="s_const"))
        s_enc = ec(nc.semaphore(name="s_enc"))
        s_encT = ec(nc.semaphore(name="s_encT"))
        s_h = ec(nc.semaphore(name="s_h"))
        s_g = ec(nc.semaphore(name="s_g"))
        s_edone = ec(nc.semaphore(name="s_edone"))
        s_x = ec(nc.semaphore(name="s_x"))
        s_pm = ec(nc.semaphore(name="s_pm"))
        s_p = ec(nc.semaphore(name="s_p"))
        s_ctx = ec(nc.semaphore(name="s_ctx"))
        s_cadd = ec(nc.semaphore(name="s_cadd"))
        s_out = ec(nc.semaphore(name="s_out"))

        block = ec(nc.Block())

        @block.sync
        def _(sync):
            sync.dma_start_transpose(whT[:, :, :], wh[:, :]).then_inc(s_const, 16)
            sync.dma_start_transpose(vt[:, :], vp[:, :]).then_inc(s_const, 16)
            sync.dma_start_transpose(sT[:, :, :], sp[:, :]).then_inc(s_const, 16)
            sync.dma_start_transpose(maskc[:, :], maskT[:, :]).then_inc(s_const, 16)
            for i, (j, b) in enumerate(tiles):
                if i >= 4:
                    sync.wait_ge(s_ctx, i - 3)  # enc_sb slot: ctx-MMs of i-4 done
                sync.dma_start_transpose(
                    enc_sbs[i % 4][:, :, :], encTT[b, :, ts(j, T_TILE)]
                ).then_inc(s_enc, 16)
                if i >= 3:
                    sync.wait_ge(s_h, 4 * (i - 3) + 4)  # encT slot: h-MMs of i-3 done
                sync.dma_start_transpose(
                    encTs[i % 3][:, :, :], enc[b, ts(j, T_TILE), :]
                ).then_inc(s_encT, 16)
            # redo loads for tiles 0 and 1 (their first-pass ctx chains are
            # re-executed at the end; see ctx redo below)
            sync.wait_ge(s_ctx, N_TILES)
            for rd in range(2):
                jj, bb = tiles[rd]
                sync.dma_start_transpose(
                    enc_sbs[rd][:, :, :], encTT[bb, :, ts(jj, T_TILE)]
                ).then_inc(s_enc, 16)
            sync.wait_ge(s_enc, 16 * (N_TILES + 2))  # xposes done before regular DMAs
            sync.wait_ge(s_pm, N_TILES)
            sync.dma_start(pcol[:, :], pcol_sb[:, :]).then_inc(s_out, 16)
            sync.wait_ge(s_cadd, N_TILES + 2)
            sync.dma_start(ctxu[0:1, :], ctxall[0:1, :B_LOC, :]).then_inc(s_out, 16)
            sync.wait_ge(s_out, 32)

        @block.tensor
        def _(tensor):
            def hgroup(i, kc):
                idx = 4 * i + kc
                if idx >= 2:
                    tensor.wait_ge(s_g, idx - 1)  # hps slot consumed by tanh(idx-2)
                mm = None
                for hc in range(N_HC):
                    mm = tensor.matmul(
                        hps[idx % 2][:, :],
                        whT[:, hc, ts(kc, 128)],
                        encTs[i % 3][:, hc, :],
                        start=(hc == 0),
                        stop=(hc == N_HC - 1),
                        skip_group_check=True,
                    )
                mm.then_inc(s_h, 1)

            def egroup(i, kc):
                idx = 4 * i + kc
                if kc == 0 and i >= 1:
                    tensor.wait_ge(s_x, i)  # e banks: exp of i-1 done
                tensor.wait_ge(s_g, idx + 1)  # g(idx) ready
                mm = None
                for sc in range(N_SC):
                    mm = tensor.matmul(
                        es[sc][:, :],
                        gs[idx % 8][:, ts(sc, 128)],
                        vt[:, kc : kc + 1],
                        start=(kc == 0),
                        stop=(kc == N_KC - 1),
                        skip_group_check=True,
                    )
                mm.then_inc(s_edone, 1)

            def ctxgroup(it):
                jj, bb = tiles[it]
                tensor.wait_ge(s_p, it + 1)       # pb(it) ready
                tensor.wait_ge(s_enc, 16 * (it + 1))  # enc_sb(it) loaded
                if it >= 1:
                    # strict alternation with the DVE adds: chains launched
                    # with less delay than this produce corrupt PSUM output
                    tensor.wait_ge(s_cadd, it)
                mm = None
                for sc in range(N_SC):
                    mm = tensor.matmul(
                        cpss[it % 2][0:1, :],
                        pbs[it % 4][:, sc : sc + 1],
                        enc_sbs[it % 4][:, sc, :],
                        start=(sc == 0),
                        stop=(sc == N_SC - 1),
                        skip_group_check=True,
                    )
                mm.then_inc(s_ctx, 1)

            tensor.wait_ge(s_const, 64)
            # warm up the ctx PSUM banks: the first M=1 chain written to a
            # cold bank reads back empty on HW; make the real chains non-first
            for k in range(2):
                tensor.matmul(
                    cpss[k][0:1, :], vt[:, 0:1], whT[:, 0, :],
                    start=True, stop=True, skip_group_check=True,
                )
            for i, (j, b) in enumerate(tiles):
                tensor.wait_ge(s_encT, 16 * (i + 1))
                for kc in range(N_KC):
                    hgroup(i, kc)
                if i >= 1:
                    for kc in range(N_KC):
                        egroup(i - 1, kc)
                if i >= 2:
                    ctxgroup(i - 2)
            for kc in range(N_KC):
                egroup(N_TILES - 1, kc)
            ctxgroup(N_TILES - 2)
            ctxgroup(N_TILES - 1)
            # dummy MM: extra s_ctx tick for the final add's drain margin
            tensor.matmul(
                hps[0][:, 0:1], whT[:, 0, 0:128], vt[:, 0:1],
                start=True, stop=True, skip_group_check=True,
            ).then_inc(s_ctx, 1)
            # redo chains for tiles 0/1: their first-pass results are
            # corrupted (earliest chains race something unidentified); chains
            # executed late are reliably correct
            for rd in range(2):
                tensor.wait_ge(s_p, N_TILES + 1 + rd)
                tensor.wait_ge(s_enc, 16 * (N_TILES + 1 + rd))
                tensor.wait_ge(s_cadd, N_TILES)  # all loop adds done
                mm = None
                for sc in range(N_SC):
                    mm = tensor.matmul(
                        cpss[rd][0:1, :],
                        pbs[rd][:, sc : sc + 1],
                        enc_sbs[rd][:, sc, :],
                        start=(sc == 0),
                        stop=(sc == N_SC - 1),
                        skip_group_check=True,
                    )
                mm.then_inc(s_ctx, 1)
            # trailing dummy for the last redo add's margin
            tensor.matmul(
                hps[0][:, 0:1], whT[:, 0, 0:128], vt[:, 0:1],
                start=True, stop=True, skip_group_check=True,
            ).then_inc(s_ctx, 1)

        @block.scalar
        def _(scalar):
            def expgroup(i):
                if i >= 2:
                    scalar.wait_ge(s_pm, i - 1)  # ptmp slot: mul of i-2 done
                scalar.wait_ge(s_edone, 4 * i + 4)
                for sc in range(N_SC):
                    inst = scalar.activation(
                        ptmps[i % 2][:, sc : sc + 1], es[sc][:, :], AF.Exp
                    )
                inst.then_inc(s_x, 1)

            scalar.wait_ge(s_const, 64)
            for i, (j, b) in enumerate(tiles):
                for kc in range(N_KC):
                    idx = 4 * i + kc
                    if idx >= 8:
                        scalar.wait_ge(s_edone, idx - 7)  # g slot: e-MMs of idx-8 done
                    scalar.wait_ge(s_h, idx + 1)
                    scalar.activation(
                        gs[idx % 8][:, :],
                        hps[idx % 2][:, :],
                        AF.Tanh,
                        bias=sT[:, kc, b : b + 1],
                        scale=1.0,
                    ).then_inc(s_g, 1)
                # exp deferred one tile: e-groups of tile i run on PE only
                # after tile i+1's h-groups, which need this tile's tanhs
                if i >= 1:
                    expgroup(i - 1)
            expgroup(N_TILES - 1)

        @block.vector
        def _(vector):
            vector.memset(ctxall[:, :, :], 0.0)
            vector.wait_ge(s_const, 64)

            def ctxadd(it):
                jj, bb = tiles[it]
                # tiles 0/1 race something at kernel start and are redone at
                # the end; park their corrupt first-pass adds in a trash row
                seg = bb if it >= 2 else B_LOC
                vector.wait_ge(s_ctx, it + 1)
                # s_ctx can fire before the chain's PSUM drain lands; also wait
                # for a later PE event (h2 of the next tile) as drain margin
                vector.wait_ge(s_h, min(4 * (it + 2) + 3, 4 * N_TILES))
                vector.tensor_add(
                    ctxall[0:1, seg, :],
                    ctxall[0:1, seg, :],
                    cpss[it % 2][0:1, :],
                ).then_inc(s_cadd, 1)

            for i, (j, b) in enumerate(tiles):
                cb = b * (N_J * N_SC) + j * N_SC
                vector.wait_ge(s_x, i + 1)
                vector.tensor_mul(
                    pcol_sb[:, cb : cb + N_SC],
                    ptmps[i % 2][:, :],
                    maskc[:, cb : cb + N_SC],
                ).then_inc(s_pm, 1)
                if i >= 2:
                    vector.wait_ge(s_ctx, i - 1)  # pb slot: ctx-MMs of i-2 done
                vector.tensor_copy(
                    out=pbs[i % 4][:, :], in_=pcol_sb[:, cb : cb + N_SC]
                )
                # s_p signalled by a trailing op: a DVE op issues only after
                # the previous op's pipe drains, so pb's writes are visible
                vector.memset(junk[0:1, :], 0.0).then_inc(s_p, 1)
                if i >= 1:
                    ctxadd(i - 1)
            ctxadd(N_TILES - 1)
            # redo: recopy p for tiles 0/1 from pcol_sb, then add the redone
            # chains (s_ctx ticks: 33 = dummy, 34/35 = redo chains, 36 = dummy)
            for rd in range(2):
                jr, br = tiles[rd]
                cbr = br * (N_J * N_SC) + jr * N_SC
                vector.tensor_copy(
                    out=pbs[rd][:, :], in_=pcol_sb[:, cbr : cbr + N_SC]
                )
                vector.memset(junk[0:1, :], 0.0).then_inc(s_p, 1)
            for rd in range(2):
                jr, br = tiles[rd]
                vector.wait_ge(s_ctx, N_TILES + 3 + rd)
                vector.tensor_add(
                    ctxall[0:1, br, :],
                    ctxall[0:1, br, :],
                    cpss[rd][0:1, :],
                ).then_inc(s_cadd, 1)

    return nc


def _prep_shared(W_h, W_s, v, dec_state):
    bf16 = ml_dtypes.bfloat16
    wh_in = np.ascontiguousarray(W_h).astype(bf16)
    vp_in = np.zeros((16, 128), dtype=bf16)
    vp_in[:N_KC] = v.reshape(N_KC, 128).astype(bf16)
    s = (dec_state.astype(np.float64) @ W_s.T.astype(np.float64)).astype(np.float32)
    return wh_in, vp_in, s


def kernel(enc_seq, enc_mask, dec_state, W_h, W_s, v):
    from concourse.bass_utils import run_bass_kernel_spmd

    enc_seq = np.asarray(enc_seq)
    enc_mask = np.asarray(enc_mask)
    dec_state = np.asarray(dec_state)
    W_h = np.asarray(W_h)
    W_s = np.asarray(W_s)
    v = np.asarray(v)
    bf16 = ml_dtypes.bfloat16

    if "nc" not in _cache:
        _cache["nc"] = _build_nc()
    nc = _cache["nc"]

    wh_in, vp_in, s = _prep_shared(W_h, W_s, v, dec_state)
    enc_bf = enc_seq.astype(bf16)                               # [B, T, H]
    encTT_bf = np.ascontiguousarray(enc_bf.transpose(0, 2, 1))  # [B, H, T]

    in_maps = []
    for core in range(N_CORES):
        bs = slice(core * B_LOC, (core + 1) * B_LOC)
        sp_in = np.zeros((16, HID), dtype=bf16)
        sp_in[:B_LOC] = s[bs].astype(bf16)
        # maskT[c, p] = mask[b, j*512 + sc*128 + p],  c = b*32 + j*4 + sc
        mk = (
            enc_mask[bs]
            .reshape(B_LOC, N_J, N_SC, 128)
            .reshape(N_COLS, 128)
            .astype(bf16)
        )
        in_maps.append(
            {
                "enc": np.ascontiguousarray(enc_bf[bs]),
                "encTT": np.ascontiguousarray(encTT_bf[bs]),
                "wh": wh_in,
                "vp": vp_in,
                "sp": sp_in,
                "maskT": np.ascontiguousarray(mk),
            }
        )

    res = run_bass_kernel_spmd(nc, in_maps, core_ids=list(range(N_CORES)))

    ctx = np.empty((B, HID), np.float32)
    a = np.empty((B, T), np.float32)
    for core in range(N_CORES):
        r = res.results[core]
        p_un = (
            r["pcol"]
            .reshape(128, B_LOC, N_J, N_SC)
            .transpose(1, 2, 3, 0)
            .reshape(B_LOC, T)
            .astype(np.float64)
        )
        denom = p_un.sum(axis=1, keepdims=True)
        bs = slice(core * B_LOC, (core + 1) * B_LOC)
        a[bs] = (p_un / denom).astype(np.float32)
        ctx[bs] = (r["ctxu"].reshape(B_LOC, HID).astype(np.float64) / denom).astype(
            np.float32
        )
    return ctx, a


# revision 39
# speedup vs baseline: 1.0885x; 1.0296x over previous
"""Additive (Bahdanau) attention on Trainium2, 8-core SPMD, data-parallel over batch.

Raw-Bass implementation (explicit semaphores; the Tile scheduler attaches >1
sync-wait command per instruction, which this walrus build cannot codegen —
raw bass emits standalone wait_ge instructions instead).

Per core (4 local batches), one streaming pass over enc_seq:
  encT tile [128h, 4hc, 512t]  <- DMA xbar transpose of enc_bf16 (DRAM)
  enc  tile [128t, 4sc, 512h]  <- regular strided DMA of enc_bf16 (DRAM)
  hT[k,t] = sum_h WhT[h,k] encT[h,t]    (bf16 MMs, f32 PSUM, 4 k-chunks)
  g = tanh(hT + s[k])                   (ACT, per-partition bias, bf16 out)
  e_col[t] = sum_k g[k,t] v[k]          (PE, g stationary, e in column form)
  p = exp(e) * mask                     (no max subtraction: |e| <= sum|v| ~ 23)
  ctx_u[b] += p_chunk.T @ enc_chunk     (PE chain per tile, DVE-accumulated)
Host: s = dec @ Ws.T pre-projection, bf16 layout prep, final softmax
normalization (a = p/sum(p), ctx = ctx_u/sum(p)), and the p de-transpose.
"""

import numpy as np
import ml_dtypes
from contextlib import ExitStack

HID = 512
T = 4096
B = 32
N_CORES = 8
B_LOC = B // N_CORES          # 4 batches per core
T_TILE = 512
N_J = T // T_TILE             # 8 t-tiles per batch
N_SC = T_TILE // 128          # 4 sub-chunks per tile
N_HC = HID // 128             # 4 h-chunks
N_KC = HID // 128             # 4 k-chunks
N_COLS = B_LOC * N_J * N_SC   # 128 p-columns per core
N_TILES = N_J * B_LOC         # 32 (b,j) tiles per core

_cache = {}


def _build_nc():
    import concourse.bass as bass
    from concourse import mybir

    f32 = mybir.dt.float32
    bf16 = mybir.dt.bfloat16
    AF = mybir.ActivationFunctionType
    ts = bass.ts

    nc = bass.Bass()
    enc = nc.dram_tensor("enc", [B_LOC, T, HID], bf16, kind="ExternalInput")
    wh = nc.dram_tensor("wh", [HID, HID], bf16, kind="ExternalInput")
    vp = nc.dram_tensor("vp", [16, 128], bf16, kind="ExternalInput")
    sp = nc.dram_tensor("sp", [16, HID], bf16, kind="ExternalInput")
    maskT = nc.dram_tensor("maskT", [N_COLS, 128], bf16, kind="ExternalInput")
    pcol = nc.dram_tensor("pcol", [128, N_COLS], f32, kind="ExternalOutput")
    ctxu = nc.dram_tensor("ctxu", [1, B_LOC * HID], f32, kind="ExternalOutput")

    tiles = [(j, b) for j in range(N_J) for b in range(B_LOC)]

    with ExitStack() as ex:
        ec = ex.enter_context
        whT = ec(nc.sbuf_tensor("whT", [128, N_HC, HID], bf16))
        vt = ec(nc.sbuf_tensor("vt", [128, 16], bf16))
        sT = ec(nc.sbuf_tensor("sT", [128, N_KC, 16], bf16))
        maskc = ec(nc.sbuf_tensor("maskc", [128, N_COLS], bf16))
        pcol_sb = ec(nc.sbuf_tensor("pcol_sb", [128, N_COLS], f32))
        ctxall = ec(nc.sbuf_tensor("ctxall", [1, B_LOC + 1, HID], f32))
        enc_sbs = [ec(nc.sbuf_tensor(f"encsb{k}", [128, N_SC, HID], bf16)) for k in range(4)]
        encTs = [ec(nc.sbuf_tensor(f"encT{k}", [128, N_HC, T_TILE], bf16)) for k in range(3)]
        gs = [ec(nc.sbuf_tensor(f"g{k}", [128, T_TILE], bf16)) for k in range(8)]
        ptmps = [ec(nc.sbuf_tensor(f"ptmp{k}", [128, N_SC], f32)) for k in range(2)]
        junk = ec(nc.sbuf_tensor("junk", [1, 4], f32))
        pbs = [ec(nc.sbuf_tensor(f"pb{k}", [128, N_SC], bf16)) for k in range(4)]

        hps = [ec(nc.psum_tensor(f"hps{k}", [128, T_TILE], f32)) for k in range(2)]
        # one PSUM bank per e-column: start=True clears accumulation state at
        # bank granularity, so per-column chains must not share a bank
        es = [ec(nc.psum_tensor(f"es{k}", [128, 1], f32)) for k in range(N_SC)]
        # ctx: single-chain group per tile (start..stop), DVE-accumulated into
        # SBUF. Two alternating banks: start=True clears state bank-wide, so
        # chain it+1 must not share a bank with chain it (whose row the DVE
        # still has to read), and DVE must never read a bank PE is writing.
        cpss = [ec(nc.psum_tensor(f"cps{k}", [128, HID], f32)) for k in range(2)]

        s_const = ec(nc.semaphore(name="s_const"))
        s_enc = ec(nc.semaphore(name="s_enc"))
        s_encT = ec(nc.semaphore(name="s_encT"))
        s_h = ec(nc.semaphore(name="s_h"))
        s_g = ec(nc.semaphore(name="s_g"))
        s_edone = ec(nc.semaphore(name="s_edone"))
        s_x = ec(nc.semaphore(name="s_x"))
        s_pm = ec(nc.semaphore(name="s_pm"))
        s_p = ec(nc.semaphore(name="s_p"))
        s_ctx = ec(nc.semaphore(name="s_ctx"))
        s_cadd = ec(nc.semaphore(name="s_cadd"))
        s_out = ec(nc.semaphore(name="s_out"))

        block = ec(nc.Block())

        @block.sync
        def _(sync):
            sync.dma_start_transpose(whT[:, :, :], wh[:, :]).then_inc(s_const, 16)
            sync.dma_start_transpose(vt[:, :], vp[:, :]).then_inc(s_const, 16)
            sync.dma_start_transpose(sT[:, :, :], sp[:, :]).then_inc(s_const, 16)
            sync.dma_start_transpose(maskc[:, :], maskT[:, :]).then_inc(s_const, 16)
            for i, (j, b) in enumerate(tiles):
                if i >= 4:
                    sync.wait_ge(s_ctx, i - 3)  # enc_sb slot: ctx-MMs of i-4 done
                sync.dma_start(
                    enc_sbs[i % 4][:, :, :],
                    enc[b, ts(j, T_TILE), :].rearrange("(sc p) h -> p sc h", p=128),
                ).then_inc(s_enc, 16)
                if i >= 3:
                    sync.wait_ge(s_h, 4 * (i - 3) + 4)  # encT slot: h-MMs of i-3 done
                sync.dma_start_transpose(
                    encTs[i % 3][:, :, :], enc[b, ts(j, T_TILE), :]
                ).then_inc(s_encT, 16)
            # redo loads for tiles 0 and 1 (their first-pass ctx chains are
            # re-executed at the end; see ctx redo below)
            sync.wait_ge(s_ctx, N_TILES)
            for rd in range(2):
                jj, bb = tiles[rd]
                sync.dma_start(
                    enc_sbs[rd][:, :, :],
                    enc[bb, ts(jj, T_TILE), :].rearrange("(sc p) h -> p sc h", p=128),
                ).then_inc(s_enc, 16)
            sync.wait_ge(s_enc, 16 * (N_TILES + 2))  # xposes done before regular DMAs
            sync.wait_ge(s_pm, N_TILES)
            sync.dma_start(pcol[:, :], pcol_sb[:, :]).then_inc(s_out, 16)
            sync.wait_ge(s_cadd, N_TILES + 2)
            sync.dma_start(ctxu[0:1, :], ctxall[0:1, :B_LOC, :]).then_inc(s_out, 16)
            sync.wait_ge(s_out, 32)

        @block.tensor
        def _(tensor):
            def hgroup(i, kc):
                idx = 4 * i + kc
                if idx >= 2:
                    tensor.wait_ge(s_g, idx - 1)  # hps slot consumed by tanh(idx-2)
                mm = None
                for hc in range(N_HC):
                    mm = tensor.matmul(
                        hps[idx % 2][:, :],
                        whT[:, hc, ts(kc, 128)],
                        encTs[i % 3][:, hc, :],
                        start=(hc == 0),
                        stop=(hc == N_HC - 1),
                        skip_group_check=True,
                    )
                mm.then_inc(s_h, 1)

            def egroup(i, kc):
                idx = 4 * i + kc
                if kc == 0 and i >= 1:
                    tensor.wait_ge(s_x, i)  # e banks: exp of i-1 done
                tensor.wait_ge(s_g, idx + 1)  # g(idx) ready
                mm = None
                for sc in range(N_SC):
                    mm = tensor.matmul(
                        es[sc][:, :],
                        gs[idx % 8][:, ts(sc, 128)],
                        vt[:, kc : kc + 1],
                        start=(kc == 0),
                        stop=(kc == N_KC - 1),
                        skip_group_check=True,
                    )
                mm.then_inc(s_edone, 1)

            def ctxgroup(it):
                jj, bb = tiles[it]
                tensor.wait_ge(s_p, it + 1)       # pb(it) ready
                tensor.wait_ge(s_enc, 16 * (it + 1))  # enc_sb(it) loaded
                if it >= 1:
                    # strict alternation with the DVE adds: chains launched
                    # with less delay than this produce corrupt PSUM output
                    tensor.wait_ge(s_cadd, it)
                mm = None
                for sc in range(N_SC):
                    mm = tensor.matmul(
                        cpss[it % 2][0:1, :],
                        pbs[it % 4][:, sc : sc + 1],
                        enc_sbs[it % 4][:, sc, :],
                        start=(sc == 0),
                        stop=(sc == N_SC - 1),
                        skip_group_check=True,
                    )
                mm.then_inc(s_ctx, 1)

            tensor.wait_ge(s_const, 64)
            # warm up the ctx PSUM banks: the first M=1 chain written to a
            # cold bank reads back empty on HW; make the real chains non-first
            for k in range(2):
                tensor.matmul(
                    cpss[k][0:1, :], vt[:, 0:1], whT[:, 0, :],
                    start=True, stop=True, skip_group_check=True,
                )
            for i, (j, b) in enumerate(tiles):
                tensor.wait_ge(s_encT, 16 * (i + 1))
                for kc in range(N_KC):
                    hgroup(i, kc)
                if i >= 1:
                    for kc in range(N_KC):
                        egroup(i - 1, kc)
                if i >= 2:
                    ctxgroup(i - 2)
            for kc in range(N_KC):
                egroup(N_TILES - 1, kc)
            ctxgroup(N_TILES - 2)
            ctxgroup(N_TILES - 1)
            # dummy MM: extra s_ctx tick for the final add's drain margin
            tensor.matmul(
                hps[0][:, 0:1], whT[:, 0, 0:128], vt[:, 0:1],
                start=True, stop=True, skip_group_check=True,
            ).then_inc(s_ctx, 1)
            # redo chains for tiles 0/1: their first-pass results are
            # corrupted (earliest chains race something unidentified); chains
            # executed late are reliably correct
            for rd in range(2):
                tensor.wait_ge(s_p, N_TILES + 1 + rd)
                tensor.wait_ge(s_enc, 16 * (N_TILES + 1 + rd))
                tensor.wait_ge(s_cadd, N_TILES)  # all loop adds done
                mm = None
                for sc in range(N_SC):
                    mm = tensor.matmul(
                        cpss[rd][0:1, :],
                        pbs[rd][:, sc : sc + 1],
                        enc_sbs[rd][:, sc, :],
                        start=(sc == 0),
                        stop=(sc == N_SC - 1),
                        skip_group_check=True,
                    )
                mm.then_inc(s_ctx, 1)
            # trailing dummy for the last redo add's margin
            tensor.matmul(
                hps[0][:, 0:1], whT[:, 0, 0:128], vt[:, 0:1],
                start=True, stop=True, skip_group_check=True,
            ).then_inc(s_ctx, 1)

        @block.scalar
        def _(scalar):
            def expgroup(i):
                if i >= 2:
                    scalar.wait_ge(s_pm, i - 1)  # ptmp slot: mul of i-2 done
                scalar.wait_ge(s_edone, 4 * i + 4)
                for sc in range(N_SC):
                    inst = scalar.activation(
                        ptmps[i % 2][:, sc : sc + 1], es[sc][:, :], AF.Exp
                    )
                inst.then_inc(s_x, 1)

            scalar.wait_ge(s_const, 64)
            for i, (j, b) in enumerate(tiles):
                for kc in range(N_KC):
                    idx = 4 * i + kc
                    if idx >= 8:
                        scalar.wait_ge(s_edone, idx - 7)  # g slot: e-MMs of idx-8 done
                    scalar.wait_ge(s_h, idx + 1)
                    scalar.activation(
                        gs[idx % 8][:, :],
                        hps[idx % 2][:, :],
                        AF.Tanh,
                        bias=sT[:, kc, b : b + 1],
                        scale=1.0,
                    ).then_inc(s_g, 1)
                # exp deferred one tile: e-groups of tile i run on PE only
                # after tile i+1's h-groups, which need this tile's tanhs
                if i >= 1:
                    expgroup(i - 1)
            expgroup(N_TILES - 1)

        @block.vector
        def _(vector):
            vector.memset(ctxall[:, :, :], 0.0)
            vector.wait_ge(s_const, 64)

            def ctxadd(it):
                jj, bb = tiles[it]
                # tiles 0/1 race something at kernel start and are redone at
                # the end; park their corrupt first-pass adds in a trash row
                seg = bb if it >= 2 else B_LOC
                vector.wait_ge(s_ctx, it + 1)
                # s_ctx can fire before the chain's PSUM drain lands; also wait
                # for a later PE event (h2 of the next tile) as drain margin
                vector.wait_ge(s_h, min(4 * (it + 2) + 3, 4 * N_TILES))
                vector.tensor_add(
                    ctxall[0:1, seg, :],
                    ctxall[0:1, seg, :],
                    cpss[it % 2][0:1, :],
                ).then_inc(s_cadd, 1)

            for i, (j, b) in enumerate(tiles):
                cb = b * (N_J * N_SC) + j * N_SC
                vector.wait_ge(s_x, i + 1)
                vector.tensor_mul(
                    pcol_sb[:, cb : cb + N_SC],
                    ptmps[i % 2][:, :],
                    maskc[:, cb : cb + N_SC],
                ).then_inc(s_pm, 1)
                if i >= 2:
                    vector.wait_ge(s_ctx, i - 1)  # pb slot: ctx-MMs of i-2 done
                vector.tensor_copy(
                    out=pbs[i % 4][:, :], in_=pcol_sb[:, cb : cb + N_SC]
                )
                # s_p signalled by a trailing op: a DVE op issues only after
                # the previous op's pipe drains, so pb's writes are visible
                vector.memset(junk[0:1, :], 0.0).then_inc(s_p, 1)
                if i >= 1:
                    ctxadd(i - 1)
            ctxadd(N_TILES - 1)
            # redo: recopy p for tiles 0/1 from pcol_sb, then add the redone
            # chains (s_ctx ticks: 33 = dummy, 34/35 = redo chains, 36 = dummy)
            for rd in range(2):
                jr, br = tiles[rd]
                cbr = br * (N_J * N_SC) + jr * N_SC
                vector.tensor_copy(
                    out=pbs[rd][:, :], in_=pcol_sb[:, cbr : cbr + N_SC]
                )
                vector.memset(junk[0:1, :], 0.0).then_inc(s_p, 1)
            for rd in range(2):
                jr, br = tiles[rd]
                vector.wait_ge(s_ctx, N_TILES + 3 + rd)
                vector.tensor_add(
                    ctxall[0:1, br, :],
                    ctxall[0:1, br, :],
                    cpss[rd][0:1, :],
                ).then_inc(s_cadd, 1)

    return nc


def _prep_shared(W_h, W_s, v, dec_state):
    bf16 = ml_dtypes.bfloat16
    wh_in = np.ascontiguousarray(W_h).astype(bf16)
    vp_in = np.zeros((16, 128), dtype=bf16)
    vp_in[:N_KC] = v.reshape(N_KC, 128).astype(bf16)
    s = (dec_state.astype(np.float64) @ W_s.T.astype(np.float64)).astype(np.float32)
    return wh_in, vp_in, s


def kernel(enc_seq, enc_mask, dec_state, W_h, W_s, v):
    from concourse.bass_utils import run_bass_kernel_spmd

    enc_seq = np.asarray(enc_seq)
    enc_mask = np.asarray(enc_mask)
    dec_state = np.asarray(dec_state)
    W_h = np.asarray(W_h)
    W_s = np.asarray(W_s)
    v = np.asarray(v)
    bf16 = ml_dtypes.bfloat16

    if "nc" not in _cache:
        _cache["nc"] = _build_nc()
    nc = _cache["nc"]

    wh_in, vp_in, s = _prep_shared(W_h, W_s, v, dec_state)
    enc_bf = enc_seq.astype(bf16)                               # [B, T, H]

    in_maps = []
    for core in range(N_CORES):
        bs = slice(core * B_LOC, (core + 1) * B_LOC)
        sp_in = np.zeros((16, HID), dtype=bf16)
        sp_in[:B_LOC] = s[bs].astype(bf16)
        # maskT[c, p] = mask[b, j*512 + sc*128 + p],  c = b*32 + j*4 + sc
        mk = (
            enc_mask[bs]
            .reshape(B_LOC, N_J, N_SC, 128)
            .reshape(N_COLS, 128)
            .astype(bf16)
        )
        in_maps.append(
            {
                "enc": np.ascontiguousarray(enc_bf[bs]),
                "wh": wh_in,
                "vp": vp_in,
                "sp": sp_in,
                "maskT": np.ascontiguousarray(mk),
            }
        )

    res = run_bass_kernel_spmd(nc, in_maps, core_ids=list(range(N_CORES)))

    ctx = np.empty((B, HID), np.float32)
    a = np.empty((B, T), np.float32)
    for core in range(N_CORES):
        r = res.results[core]
        p_un = (
            r["pcol"]
            .reshape(128, B_LOC, N_J, N_SC)
            .transpose(1, 2, 3, 0)
            .reshape(B_LOC, T)
            .astype(np.float64)
        )
        denom = p_un.sum(axis=1, keepdims=True)
        bs = slice(core * B_LOC, (core + 1) * B_LOC)
        a[bs] = (p_un / denom).astype(np.float32)
        ctx[bs] = (r["ctxu"].reshape(B_LOC, HID).astype(np.float64) / denom).astype(
            np.float32
        )
    return ctx, a


# revision 40
# speedup vs baseline: 1.1970x; 1.0996x over previous
"""Additive (Bahdanau) attention on Trainium2, 8-core SPMD, data-parallel over batch.

Raw-Bass implementation (explicit semaphores; the Tile scheduler attaches >1
sync-wait command per instruction, which this walrus build cannot codegen —
raw bass emits standalone wait_ge instructions instead).

Per core (4 local batches), one streaming pass over enc_seq:
  encT tile [128h, 4hc, 512t]  <- DMA xbar transpose of enc_bf16 (DRAM)
  enc  tile [128t, 4sc, 512h]  <- regular strided DMA of enc_bf16 (DRAM)
  hT[k,t] = sum_h WhT[h,k] encT[h,t]    (bf16 MMs, f32 PSUM, 4 k-chunks)
  g = tanh(hT + s[k])                   (ACT, per-partition bias, bf16 out)
  e_col[t] = sum_k g[k,t] v[k]          (PE, g stationary, e in column form)
  p = exp(e) * mask                     (no max subtraction: |e| <= sum|v| ~ 23)
  ctx_u[b] += p_chunk.T @ enc_chunk     (PE chain per tile, DVE-accumulated)
Host: s = dec @ Ws.T pre-projection, bf16 layout prep, final softmax
normalization (a = p/sum(p), ctx = ctx_u/sum(p)), and the p de-transpose.
"""

import numpy as np
import ml_dtypes
from contextlib import ExitStack

HID = 512
T = 4096
B = 32
N_CORES = 8
B_LOC = B // N_CORES          # 4 batches per core
T_TILE = 512
N_J = T // T_TILE             # 8 t-tiles per batch
N_SC = T_TILE // 128          # 4 sub-chunks per tile
N_HC = HID // 128             # 4 h-chunks
N_KC = HID // 128             # 4 k-chunks
N_COLS = B_LOC * N_J * N_SC   # 128 p-columns per core
N_TILES = N_J * B_LOC         # 32 (b,j) tiles per core

_cache = {}


def _build_nc():
    import concourse.bass as bass
    from concourse import mybir

    f32 = mybir.dt.float32
    bf16 = mybir.dt.bfloat16
    AF = mybir.ActivationFunctionType
    ts = bass.ts

    nc = bass.Bass()
    enc = nc.dram_tensor("enc", [B_LOC, T, HID], bf16, kind="ExternalInput")
    wh = nc.dram_tensor("wh", [HID, HID], bf16, kind="ExternalInput")
    vp = nc.dram_tensor("vp", [16, 128], bf16, kind="ExternalInput")
    sp = nc.dram_tensor("sp", [16, HID], bf16, kind="ExternalInput")
    maskT = nc.dram_tensor("maskT", [N_COLS, 128], bf16, kind="ExternalInput")
    pcol = nc.dram_tensor("pcol", [128, N_COLS], f32, kind="ExternalOutput")
    ctxu = nc.dram_tensor("ctxu", [1, B_LOC * HID], f32, kind="ExternalOutput")

    tiles = [(j, b) for j in range(N_J) for b in range(B_LOC)]

    with ExitStack() as ex:
        ec = ex.enter_context
        whT = ec(nc.sbuf_tensor("whT", [128, N_HC, HID], bf16))
        vt = ec(nc.sbuf_tensor("vt", [128, 16], bf16))
        sT = ec(nc.sbuf_tensor("sT", [128, N_KC, 16], bf16))
        maskc = ec(nc.sbuf_tensor("maskc", [128, N_COLS], bf16))
        pcol_sb = ec(nc.sbuf_tensor("pcol_sb", [128, N_COLS], f32))
        ctxall = ec(nc.sbuf_tensor("ctxall", [1, B_LOC + 1, HID], f32))
        enc_sbs = [ec(nc.sbuf_tensor(f"encsb{k}", [128, N_SC, HID], bf16)) for k in range(8)]
        encTs = [ec(nc.sbuf_tensor(f"encT{k}", [128, N_HC, T_TILE], bf16)) for k in range(3)]
        gs = [ec(nc.sbuf_tensor(f"g{k}", [128, T_TILE], bf16)) for k in range(8)]
        ptmps = [ec(nc.sbuf_tensor(f"ptmp{k}", [128, N_SC], f32)) for k in range(2)]
        junk = ec(nc.sbuf_tensor("junk", [1, 4], f32))
        pbs = [ec(nc.sbuf_tensor(f"pb{k}", [128, N_SC], bf16)) for k in range(4)]

        hps = [ec(nc.psum_tensor(f"hps{k}", [128, T_TILE], f32)) for k in range(2)]
        # one PSUM bank per e-column: start=True clears accumulation state at
        # bank granularity, so per-column chains must not share a bank
        es = [ec(nc.psum_tensor(f"es{k}", [128, 1], f32)) for k in range(N_SC)]
        # ctx: single-chain group per tile (start..stop), DVE-accumulated into
        # SBUF. Two alternating banks: start=True clears state bank-wide, so
        # chain it+1 must not share a bank with chain it (whose row the DVE
        # still has to read), and DVE must never read a bank PE is writing.
        cpss = [ec(nc.psum_tensor(f"cps{k}", [128, HID], f32)) for k in range(2)]

        s_const = ec(nc.semaphore(name="s_const"))
        s_enc = ec(nc.semaphore(name="s_enc"))
        s_encT = ec(nc.semaphore(name="s_encT"))
        s_h = ec(nc.semaphore(name="s_h"))
        s_g = ec(nc.semaphore(name="s_g"))
        s_edone = ec(nc.semaphore(name="s_edone"))
        s_x = ec(nc.semaphore(name="s_x"))
        s_pm = ec(nc.semaphore(name="s_pm"))
        s_p = ec(nc.semaphore(name="s_p"))
        s_ctx = ec(nc.semaphore(name="s_ctx"))
        s_cadd = ec(nc.semaphore(name="s_cadd"))
        s_out = ec(nc.semaphore(name="s_out"))

        block = ec(nc.Block())

        @block.sync
        def _(sync):
            sync.dma_start_transpose(whT[:, :, :], wh[:, :]).then_inc(s_const, 16)
            sync.dma_start_transpose(vt[:, :], vp[:, :]).then_inc(s_const, 16)
            sync.dma_start_transpose(sT[:, :, :], sp[:, :]).then_inc(s_const, 16)
            sync.dma_start_transpose(maskc[:, :], maskT[:, :]).then_inc(s_const, 16)
            for i, (j, b) in enumerate(tiles):
                if i >= 3:
                    sync.wait_ge(s_h, 4 * (i - 3) + 4)  # encT slot: h-MMs of i-3 done
                sync.dma_start_transpose(
                    encTs[i % 3][:, :, :], enc[b, ts(j, T_TILE), :]
                ).then_inc(s_encT, 16)
            # redo loads for tiles 0 and 1 (their first-pass ctx chains are
            # re-executed at the end; see ctx redo below)
            sync.wait_ge(s_ctx, N_TILES)
            for rd in range(2):
                jj, bb = tiles[rd]
                sync.dma_start(
                    enc_sbs[rd][:, :, :],
                    enc[bb, ts(jj, T_TILE), :].rearrange("(sc p) h -> p sc h", p=128),
                ).then_inc(s_enc, 16)
            sync.wait_ge(s_enc, 16 * (N_TILES + 2))  # xposes done before regular DMAs
            sync.wait_ge(s_pm, N_TILES)
            sync.dma_start(pcol[:, :], pcol_sb[:, :]).then_inc(s_out, 16)
            sync.wait_ge(s_cadd, N_TILES + 2)
            sync.dma_start(ctxu[0:1, :], ctxall[0:1, :B_LOC, :]).then_inc(s_out, 16)
            sync.wait_ge(s_out, 32)

        @block.tensor
        def _(tensor):
            def hgroup(i, kc):
                idx = 4 * i + kc
                if idx >= 2:
                    tensor.wait_ge(s_g, idx - 1)  # hps slot consumed by tanh(idx-2)
                mm = None
                for hc in range(N_HC):
                    mm = tensor.matmul(
                        hps[idx % 2][:, :],
                        whT[:, hc, ts(kc, 128)],
                        encTs[i % 3][:, hc, :],
                        start=(hc == 0),
                        stop=(hc == N_HC - 1),
                        skip_group_check=True,
                    )
                mm.then_inc(s_h, 1)

            def egroup(i, kc):
                idx = 4 * i + kc
                if kc == 0 and i >= 1:
                    tensor.wait_ge(s_x, i)  # e banks: exp of i-1 done
                tensor.wait_ge(s_g, idx + 1)  # g(idx) ready
                mm = None
                for sc in range(N_SC):
                    mm = tensor.matmul(
                        es[sc][:, :],
                        gs[idx % 8][:, ts(sc, 128)],
                        vt[:, kc : kc + 1],
                        start=(kc == 0),
                        stop=(kc == N_KC - 1),
                        skip_group_check=True,
                    )
                mm.then_inc(s_edone, 1)

            def ctxgroup(it):
                jj, bb = tiles[it]
                tensor.wait_ge(s_p, it + 1)       # pb(it) ready
                tensor.wait_ge(s_enc, 16 * (it + 1))  # enc_sb(it) loaded
                if it >= 1:
                    # strict alternation with the DVE adds: chains launched
                    # with less delay than this produce corrupt PSUM output
                    tensor.wait_ge(s_cadd, it)
                mm = None
                for sc in range(N_SC):
                    mm = tensor.matmul(
                        cpss[it % 2][0:1, :],
                        pbs[it % 4][:, sc : sc + 1],
                        enc_sbs[it % 8][:, sc, :],
                        start=(sc == 0),
                        stop=(sc == N_SC - 1),
                        skip_group_check=True,
                    )
                mm.then_inc(s_ctx, 1)

            tensor.wait_ge(s_const, 64)
            # warm up the ctx PSUM banks: the first M=1 chain written to a
            # cold bank reads back empty on HW; make the real chains non-first
            for k in range(2):
                tensor.matmul(
                    cpss[k][0:1, :], vt[:, 0:1], whT[:, 0, :],
                    start=True, stop=True, skip_group_check=True,
                )
            for i, (j, b) in enumerate(tiles):
                tensor.wait_ge(s_encT, 16 * (i + 1))
                for kc in range(N_KC):
                    hgroup(i, kc)
                if i >= 1:
                    for kc in range(N_KC):
                        egroup(i - 1, kc)
                if i >= 2:
                    ctxgroup(i - 2)
            for kc in range(N_KC):
                egroup(N_TILES - 1, kc)
            ctxgroup(N_TILES - 2)
            ctxgroup(N_TILES - 1)
            # dummy MM: extra s_ctx tick for the final add's drain margin
            tensor.matmul(
                hps[0][:, 0:1], whT[:, 0, 0:128], vt[:, 0:1],
                start=True, stop=True, skip_group_check=True,
            ).then_inc(s_ctx, 1)
            # redo chains for tiles 0/1: their first-pass results are
            # corrupted (earliest chains race something unidentified); chains
            # executed late are reliably correct
            for rd in range(2):
                tensor.wait_ge(s_p, N_TILES + 1 + rd)
                tensor.wait_ge(s_enc, 16 * (N_TILES + 1 + rd))
                tensor.wait_ge(s_cadd, N_TILES)  # all loop adds done
                mm = None
                for sc in range(N_SC):
                    mm = tensor.matmul(
                        cpss[rd][0:1, :],
                        pbs[rd][:, sc : sc + 1],
                        enc_sbs[rd][:, sc, :],
                        start=(sc == 0),
                        stop=(sc == N_SC - 1),
                        skip_group_check=True,
                    )
                mm.then_inc(s_ctx, 1)
            # trailing dummy for the last redo add's margin
            tensor.matmul(
                hps[0][:, 0:1], whT[:, 0, 0:128], vt[:, 0:1],
                start=True, stop=True, skip_group_check=True,
            ).then_inc(s_ctx, 1)

        @block.scalar
        def _(scalar):
            def expgroup(i):
                if i >= 2:
                    scalar.wait_ge(s_pm, i - 1)  # ptmp slot: mul of i-2 done
                scalar.wait_ge(s_edone, 4 * i + 4)
                for sc in range(N_SC):
                    inst = scalar.activation(
                        ptmps[i % 2][:, sc : sc + 1], es[sc][:, :], AF.Exp
                    )
                inst.then_inc(s_x, 1)

            scalar.wait_ge(s_const, 64)
            for i, (j, b) in enumerate(tiles):
                # enc_sb loads ride the second HWDGE ring (ACT queue) so the
                # two DMA streams overlap; slot wait is long satisfied here
                if i >= 8:
                    scalar.wait_ge(s_ctx, i - 7)
                scalar.dma_start(
                    enc_sbs[i % 8][:, :, :],
                    enc[b, ts(j, T_TILE), :].rearrange("(sc p) h -> p sc h", p=128),
                ).then_inc(s_enc, 16)
                for kc in range(N_KC):
                    idx = 4 * i + kc
                    if idx >= 8:
                        scalar.wait_ge(s_edone, idx - 7)  # g slot: e-MMs of idx-8 done
                    scalar.wait_ge(s_h, idx + 1)
                    scalar.activation(
                        gs[idx % 8][:, :],
                        hps[idx % 2][:, :],
                        AF.Tanh,
                        bias=sT[:, kc, b : b + 1],
                        scale=1.0,
                    ).then_inc(s_g, 1)
                # exp deferred one tile: e-groups of tile i run on PE only
                # after tile i+1's h-groups, which need this tile's tanhs
                if i >= 1:
                    expgroup(i - 1)
            expgroup(N_TILES - 1)

        @block.vector
        def _(vector):
            vector.memset(ctxall[:, :, :], 0.0)
            vector.wait_ge(s_const, 64)

            def ctxadd(it):
                jj, bb = tiles[it]
                # tiles 0/1 race something at kernel start and are redone at
                # the end; park their corrupt first-pass adds in a trash row
                seg = bb if it >= 2 else B_LOC
                vector.wait_ge(s_ctx, it + 1)
                # s_ctx can fire before the chain's PSUM drain lands; also wait
                # for a later PE event (h2 of the next tile) as drain margin
                vector.wait_ge(s_h, min(4 * (it + 2) + 3, 4 * N_TILES))
                vector.tensor_add(
                    ctxall[0:1, seg, :],
                    ctxall[0:1, seg, :],
                    cpss[it % 2][0:1, :],
                ).then_inc(s_cadd, 1)

            for i, (j, b) in enumerate(tiles):
                cb = b * (N_J * N_SC) + j * N_SC
                vector.wait_ge(s_x, i + 1)
                vector.tensor_mul(
                    pcol_sb[:, cb : cb + N_SC],
                    ptmps[i % 2][:, :],
                    maskc[:, cb : cb + N_SC],
                ).then_inc(s_pm, 1)
                if i >= 2:
                    vector.wait_ge(s_ctx, i - 1)  # pb slot: ctx-MMs of i-2 done
                vector.tensor_copy(
                    out=pbs[i % 4][:, :], in_=pcol_sb[:, cb : cb + N_SC]
                )
                # s_p signalled by a trailing op: a DVE op issues only after
                # the previous op's pipe drains, so pb's writes are visible
                vector.memset(junk[0:1, :], 0.0).then_inc(s_p, 1)
                if i >= 1:
                    ctxadd(i - 1)
            ctxadd(N_TILES - 1)
            # redo: recopy p for tiles 0/1 from pcol_sb, then add the redone
            # chains (s_ctx ticks: 33 = dummy, 34/35 = redo chains, 36 = dummy)
            for rd in range(2):
                jr, br = tiles[rd]
                cbr = br * (N_J * N_SC) + jr * N_SC
                vector.tensor_copy(
                    out=pbs[rd][:, :], in_=pcol_sb[:, cbr : cbr + N_SC]
                )
                vector.memset(junk[0:1, :], 0.0).then_inc(s_p, 1)
            for rd in range(2):
                jr, br = tiles[rd]
                vector.wait_ge(s_ctx, N_TILES + 3 + rd)
                vector.tensor_add(
                    ctxall[0:1, br, :],
                    ctxall[0:1, br, :],
                    cpss[rd][0:1, :],
                ).then_inc(s_cadd, 1)

    return nc


def _prep_shared(W_h, W_s, v, dec_state):
    bf16 = ml_dtypes.bfloat16
    wh_in = np.ascontiguousarray(W_h).astype(bf16)
    vp_in = np.zeros((16, 128), dtype=bf16)
    vp_in[:N_KC] = v.reshape(N_KC, 128).astype(bf16)
    s = (dec_state.astype(np.float64) @ W_s.T.astype(np.float64)).astype(np.float32)
    return wh_in, vp_in, s


def kernel(enc_seq, enc_mask, dec_state, W_h, W_s, v):
    from concourse.bass_utils import run_bass_kernel_spmd

    enc_seq = np.asarray(enc_seq)
    enc_mask = np.asarray(enc_mask)
    dec_state = np.asarray(dec_state)
    W_h = np.asarray(W_h)
    W_s = np.asarray(W_s)
    v = np.asarray(v)
    bf16 = ml_dtypes.bfloat16

    if "nc" not in _cache:
        _cache["nc"] = _build_nc()
    nc = _cache["nc"]

    wh_in, vp_in, s = _prep_shared(W_h, W_s, v, dec_state)
    enc_bf = enc_seq.astype(bf16)                               # [B, T, H]

    in_maps = []
    for core in range(N_CORES):
        bs = slice(core * B_LOC, (core + 1) * B_LOC)
        sp_in = np.zeros((16, HID), dtype=bf16)
        sp_in[:B_LOC] = s[bs].astype(bf16)
        # maskT[c, p] = mask[b, j*512 + sc*128 + p],  c = b*32 + j*4 + sc
        mk = (
            enc_mask[bs]
            .reshape(B_LOC, N_J, N_SC, 128)
            .reshape(N_COLS, 128)
            .astype(bf16)
        )
        in_maps.append(
            {
                "enc": np.ascontiguousarray(enc_bf[bs]),
                "wh": wh_in,
                "vp": vp_in,
                "sp": sp_in,
                "maskT": np.ascontiguousarray(mk),
            }
        )

    res = run_bass_kernel_spmd(nc, in_maps, core_ids=list(range(N_CORES)))

    ctx = np.empty((B, HID), np.float32)
    a = np.empty((B, T), np.float32)
    for core in range(N_CORES):
        r = res.results[core]
        p_un = (
            r["pcol"]
            .reshape(128, B_LOC, N_J, N_SC)
            .transpose(1, 2, 3, 0)
            .reshape(B_LOC, T)
            .astype(np.float64)
        )
        denom = p_un.sum(axis=1, keepdims=True)
        bs = slice(core * B_LOC, (core + 1) * B_LOC)
        a[bs] = (p_un / denom).astype(np.float32)
        ctx[bs] = (r["ctxu"].reshape(B_LOC, HID).astype(np.float64) / denom).astype(
            np.float32
        )
    return ctx, a
